# revision 15
# baseline (speedup 1.0000x reference)
"""Trainium2 Bass kernel for nn_BottlenectedAttention.

Algorithmic reduction (same as previous version): the reference consumes only
rows m=0 and m=1029 of the attention output, so per batch the whole attention
collapses to 20 logit columns ((ms, h) pairs) against an effective query
matrix  wq_eff[b] = Wq_h @ k_sel[b] / sqrt(DK)  of shape [E, 20], followed by
ctx[b, pair, :] = softmax_n(logits) @ feats[b]  and O(1)-sized host math.

This version is a ground-up rewrite of the device schedule:

* raw bass Block (no TileContext) -> no multi-round drain/EVSEM barrier tail
  (the old kernel spent ~8.7us there) and no per-instruction scheduling slack.
* Host pre-packs every tensor in its exact SBUF layout (feats^T for the logits
  matmul, natural feats for the context matmul, pos-encoding folded in) so
  each input is ONE large contiguous-per-partition DMA (2KB lines) instead of
  thousands of 512B descriptors.
* The 4 batches are packed into the 4 column-groups of the PE array
  (tile_position=(0, 32b)), so logits and context matmuls for all batches run
  concurrently in the systolic array.
* exp + per-row softmax denominators in one ScalarE activation (accum_out).
* Only 2 PE transposes (p -> p^T) + 1 DVE copy; context matmul streams the
  natural-layout feats directly.

Sharding: sequence dim, 256 rows per core x 8 cores; rows [2048, 2054) are a
host-side 9th flash shard (exactly as before).
"""
import sys

sys.path.insert(0, "/opt/trn_rl_repo")

import numpy as np

import concourse.bass as bass
import concourse.bacc as bacc
from concourse import mybir
from concourse.bass_utils import run_bass_kernel_spmd

E, HID, NH, DK, BTNK = 512, 640, 10, 64, 4
B, LA, LV = 4, 1024, 1024
L = LA + 1 + BTNK + LV + 1          # 2054
NPAIR = 2 * NH                       # 20 (ms, h) pairs per batch
MPAD = 32                            # col-group stride (batch b -> cols 32b..)
NCORES = 8
NSL = 256                            # per-core slice width
NKT = E // 128                       # 4 k-tiles over the embedding dim
NNB = NSL // 128                     # 2 n-tiles of 128
WARM_MM = 6                          # HAM warmup matmuls during input DMA
SLICES = [(c * NSL, c * NSL + NSL) for c in range(NCORES)]

F32 = mybir.dt.float32
BF16 = mybir.dt.bfloat16


def _pos_encoding(Ln, d):
    pos = np.arange(Ln, dtype=np.float32)[:, None]
    div = np.exp(np.arange(0, d, 2, dtype=np.float32) * (-np.log(10000.0) / d))
    pe = np.zeros((Ln, d), dtype=np.float32)
    pe[:, 0::2] = np.sin(pos * div).astype(np.float32)
    pe[:, 1::2] = np.cos(pos * div).astype(np.float32)
    return pe


def build_program():
    nc = bacc.Bacc()

    # DRAM parameters (per core).  Layouts are exactly the SBUF layouts so
    # every DMA is [128 partitions x large contiguous line].
    const_d = nc.declare_dram_parameter("constd", [128, NKT + 1, 128], BF16,
                                        isOutput=False)
    ftT_d = nc.declare_dram_parameter("ftT", [NKT, 128, B, NSL], BF16,
                                      isOutput=False)
    fN_d = nc.declare_dram_parameter("fN", [B, 128, NNB, E], BF16,
                                     isOutput=False)
    out_d = nc.declare_dram_parameter("octx", [128, E + 1], BF16,
                                      isOutput=True)

    from contextlib import ExitStack
    with ExitStack() as st:
        ec = st.enter_context
        # SBUF
        const_s = ec(nc.sbuf_tensor("const_s", [128, NKT + 1, 128], BF16))
        ftT_s = ec(nc.sbuf_tensor("ftT_s", [128, NKT, B, NSL], BF16))
        fN_s = ec(nc.sbuf_tensor("fN_s", [128, B, NNB, E], BF16))
        p_s = ec(nc.sbuf_tensor("p_s", [128, NSL], BF16))
        pT_s = ec(nc.sbuf_tensor("pT_s", [128, NSL], BF16))
        s_s = ec(nc.sbuf_tensor("s_s", [128, 1], F32))
        scr_s = ec(nc.sbuf_tensor("scr_s", [128, 1], F32))
        octx_s = ec(nc.sbuf_tensor("octx_s", [128, E + 1], BF16))
        warm_s = ec(nc.sbuf_tensor("warm_s", [128, E], BF16))
        # PSUM (bank granular allocations)
        ps_w = ec(nc.psum_tensor("ps_w", [128, E], F32))
        ps_l = ec(nc.psum_tensor("ps_l", [128, E], F32))      # use [:, :NSL]
        ps_t = ec(nc.psum_tensor("ps_t", [128, 2 * NSL], BF16))  # use [:, :NSL]
        ps_c = ec(nc.psum_tensor("ps_c", [128, E], F32))
        # semaphores (one per DMA chunk: completions are out-of-order across
        # queues, so a shared counter cannot identify WHICH chunk landed)
        sem_const = ec(nc.semaphore("sem_const"))
        sem_ft = [ec(nc.semaphore(f"sem_ft{i}")) for i in range(NKT)]
        sem_fn = [ec(nc.semaphore(f"sem_fn{i}")) for i in range(B)]
        osem = ec(nc.semaphore("osem"))   # output DMA landed
        vsem = ec(nc.semaphore("vsem"))   # vector milestones
        esem = ec(nc.semaphore("esem"))   # scalar: exp done
        tsem = ec(nc.semaphore("tsem"))   # PE milestones
        csem = ec(nc.semaphore("csem"))   # ctx psum->sbuf copies

        ident = const_s[:, NKT, :]

        with nc.Block("bk") as block:

            @block.sync
            def _(sync):
                sync.dma_start(out=const_s[:], in_=const_d[:]).then_inc(sem_const, 16)
                for et in range(NKT):
                    sync.dma_start(out=ftT_s[:, et],
                                   in_=ftT_d[et]).then_inc(sem_ft[et], 16)
                # output: wait for the ctx copy (s column written earlier)
                sync.wait_ge(csem, 1)
                sync.dma_start(out=out_d[:], in_=octx_s[:]).then_inc(osem, 16)
                # ensure the output DMA has landed before the queue retires
                sync.wait_ge(osem, 16)

            @block.gpsimd
            def _(gpsimd):
                # natural-layout feats stream AFTER ftT so ftT gets full BW
                gpsimd.wait_ge(sem_const, 16)
                for et in range(NKT):
                    gpsimd.wait_ge(sem_ft[et], 16)
                for b in range(B):
                    gpsimd.dma_start(out=fN_s[:, b],
                                     in_=fN_d[b]).then_inc(sem_fn[b], 16)

            @block.tensor
            def _(tensor):
                # HAM warmup while inputs stream
                tensor.wait_ge(vsem, 1)
                for _ in range(WARM_MM):
                    tensor.matmul(ps_w[:, :E], warm_s[:, :128], warm_s[:, :E],
                                  start=True, stop=True)
                # logits: batches in col-groups, accumulate over k-tiles
                tensor.wait_ge(sem_const, 16)
                for et in range(NKT):
                    tensor.wait_ge(sem_ft[et], 16)
                    for b in range(B):
                        mm = tensor.matmul(
                            ps_l[32 * b:32 * b + MPAD, :NSL],
                            const_s[:, et, 32 * b:32 * b + MPAD],
                            ftT_s[:, et, b, :],
                            start=(et == 0), stop=(et == NKT - 1),
                            tile_position=(0, 32 * b),
                            skip_group_check=True,
                        )
                mm.then_inc(tsem, 1)                      # logits done -> 1
                # p^T via PE transposes
                tensor.wait_ge(esem, 1)
                tensor.transpose(ps_t[:, 0:128], p_s[:, 0:128], ident)
                tensor.transpose(ps_t[:, 128:256], p_s[:, 128:256],
                                 ident).then_inc(tsem, 1)  # -> 2
                # ctx: batches in col-groups, accumulate over n-tiles
                tensor.wait_ge(vsem, 2)
                for b in range(B):
                    tensor.wait_ge(sem_fn[b], 16)
                    for nb in range(NNB):
                        mm = tensor.matmul(
                            ps_c[32 * b:32 * b + MPAD, :],
                            pT_s[:, 128 * nb + 32 * b:128 * nb + 32 * b + MPAD],
                            fN_s[:, b, nb, :],
                            start=(nb == 0), stop=(nb == NNB - 1),
                            tile_position=(0, 32 * b),
                            skip_group_check=True,
                        )
                mm.then_inc(tsem, 1)                      # ctx done -> 3

            @block.scalar
            def _(scalar):
                # dummy activation first so the act-table load happens at t=0
                scalar.activation(out=scr_s[:1, :], in_=scr_s[:1, :],
                                  func=mybir.ActivationFunctionType.Exp,
                                  bias=0.0, scale=0.0)
                scalar.wait_ge(tsem, 1)
                scalar.activation(out=p_s[:], in_=ps_l[:, :NSL],
                                  func=mybir.ActivationFunctionType.Exp,
                                  bias=0.0, scale=1.0,
                                  accum_out=s_s[:]).then_inc(esem, 1)
                # NOTE: no second PSUM reader here — concurrent DVE+ACT reads
                # of the same PSUM bank hang the core (Tile serializes these;
                # raw bass must simply avoid them).

            @block.vector
            def _(vector):
                vector.memset(warm_s[:], 0.0).then_inc(vsem, 1)
                vector.wait_ge(esem, 1)
                vector.tensor_copy(out=octx_s[:, E:E + 1], in_=s_s[:])
                vector.wait_ge(tsem, 2)
                vector.tensor_copy(out=pT_s[:], in_=ps_t[:, :NSL]).then_inc(vsem, 1)
                vector.wait_ge(tsem, 3)
                vector.tensor_copy(out=octx_s[:, 0:E],
                                   in_=ps_c[:, 0:E]).then_inc(csem, 1)

    nc.finalize()
    return nc


def _install_ntff_hook():
    """The agent image's antenv lacks axon_hooks; recreate it and register the
    ctypes NTFF profile hook against the injected libaxon_pjrt.so so that
    run_bass_kernel_spmd(trace=True) can capture HW exec times."""
    import contextlib
    import ctypes
    import types

    if "antenv.axon_hooks" in sys.modules:
        return
    mod = types.ModuleType("antenv.axon_hooks")
    state = {"hook": None}
    mod.set_axon_ntff_profile_hook = lambda h: state.__setitem__("hook", h)
    mod.get_axon_ntff_profile_hook = lambda: state["hook"]
    sys.modules["antenv.axon_hooks"] = mod
    try:
        import antenv

        antenv.axon_hooks = mod
    except ImportError:
        pass

    so_path = "/opt/axon/libaxon_pjrt.so"
    try:
        lib = ctypes.CDLL(so_path)
    except OSError:
        return
    if not hasattr(lib, "axon_start_nrt_profile"):
        return
    lib.axon_start_nrt_profile.argtypes = [
        ctypes.POINTER(ctypes.c_int64),
        ctypes.c_size_t,
    ]
    lib.axon_start_nrt_profile.restype = ctypes.c_int64
    lib.axon_stop_nrt_profile.argtypes = [ctypes.c_char_p]
    lib.axon_stop_nrt_profile.restype = ctypes.c_int64

    @contextlib.contextmanager
    def _hook(output_dir, device_ids):
        import jax

        jax.devices()
        if device_ids:
            ids = (ctypes.c_int64 * len(device_ids))(*device_ids)
            rc = lib.axon_start_nrt_profile(ids, len(device_ids))
        else:
            rc = lib.axon_start_nrt_profile(None, 0)
        if rc != 0:
            raise RuntimeError(f"axon_start_nrt_profile rc={rc}")
        try:
            yield
        finally:
            n = lib.axon_stop_nrt_profile(str(output_dir).encode())
            print(f"profile: {n} file(s) written to {output_dir}", file=sys.stderr)

    state["hook"] = _hook


_CACHE = {}


def _get_program():
    if "raw" not in _CACHE:
        _CACHE["raw"] = build_program()
    return _CACHE["raw"]


def _prepare_host(inputs):
    import ml_dtypes

    bf = ml_dtypes.bfloat16
    af = np.ascontiguousarray(np.asarray(inputs["audio_feat"], dtype=np.float32))
    vf = np.ascontiguousarray(np.asarray(inputs["video_feat"], dtype=np.float32))
    at = np.asarray(inputs["audio_tok"], dtype=np.float32)
    vt = np.asarray(inputs["video_tok"], dtype=np.float32)
    bt = np.asarray(inputs["btnk_toks"], dtype=np.float32)
    Wk = np.asarray(inputs["Wk"], dtype=np.float32)
    bk = np.asarray(inputs["bk"], dtype=np.float32)
    Wq = np.asarray(inputs["Wq"], dtype=np.float32)

    pe = _pos_encoding(L, E)

    raw = np.empty((B, L, E), np.float32)
    raw[:, :LA] = af
    raw[:, LA] = at[0, 0]
    raw[:, LA + 1:LA + 1 + BTNK] = bt[0]
    raw[:, LA + 1 + BTNK:LA + 1 + BTNK + LV] = vf
    raw[:, L - 1] = vt[0, 0]

    featsb = (raw + pe[None]).astype(bf)                     # [B, L, E]

    # effective query vectors (f64 host math, exactly as before)
    f_rows = np.stack([raw[:, 0] + pe[0], raw[:, LA + 1 + BTNK] + pe[LA + 1 + BTNK]],
                      axis=1).astype(np.float64)             # [B,2,E]
    k_sel = (f_rows @ Wk.astype(np.float64) + bk).reshape(B, 2, NH, DK)
    Wq_h = Wq.astype(np.float64).reshape(E, NH, DK)
    wq_eff = np.einsum("dhx,bmhx->bdmh", Wq_h, k_sel).reshape(B, E, NPAIR)
    wq_eff = wq_eff / np.sqrt(DK)                            # [B,E,20] f64

    # const tensor: [128, NKT+1, 128]; [:, :NKT, 32b+j] = wq, [:, NKT, :] = I
    wq_pad = np.zeros((B, E, MPAD), np.float32)
    wq_pad[:, :, :NPAIR] = wq_eff.astype(np.float32)
    const_np = np.zeros((128, NKT + 1, 128), np.float32)
    # [b, et*128+p, j] -> [p, et, 32b+j]
    const_np[:, :NKT, :] = (
        wq_pad.reshape(B, NKT, 128, MPAD)
        .transpose(2, 1, 0, 3)
        .reshape(128, NKT, B * MPAD)
    )
    const_np[:, NKT, :] = np.eye(128, dtype=np.float32)
    const_np = np.ascontiguousarray(const_np).astype(bf)

    in_maps = []
    for c, (n0, n1) in enumerate(SLICES):
        block = featsb[:, n0:n1, :]                          # [B,NSL,E] bf16
        ftT = np.ascontiguousarray(
            block.transpose(2, 0, 1).reshape(NKT, 128, B, NSL))
        fN = np.ascontiguousarray(
            block.reshape(B, NNB, 128, E).transpose(0, 2, 1, 3))
        in_maps.append({"constd": const_np, "ftT": ftT, "fN": fN})

    # host 9th flash shard for rows [2048, L)
    n0 = NCORES * NSL
    tail = featsb[:, n0:L].astype(np.float64)                # [B,6,E]
    tail_logits = np.einsum("bnd,bdp->bnp", tail, wq_eff)
    m9 = tail_logits.max(axis=1)                             # [B,20]
    p9 = np.exp(tail_logits - m9[:, None, :])
    s9 = p9.sum(axis=1)                                      # [B,20]
    p9 = p9.astype(bf).astype(np.float64)
    ctx9 = np.einsum("bnp,bnd->bpd", p9, tail)               # [B,20,E]
    return in_maps, (m9, s9, ctx9)


def _finalize(inputs, ctxs, stats, tail_partial):
    """ctxs: [8,B,20,E] unnormalized local contexts; stats: [8,B,20,2] (m, s);
    tail_partial: host-computed 9th shard for rows [2048, 2054)."""
    Wv = np.asarray(inputs["Wv"], dtype=np.float64)
    bv = np.asarray(inputs["bv"], dtype=np.float64)
    ln_g = np.asarray(inputs["ln_g"], dtype=np.float64)
    ln_b = np.asarray(inputs["ln_b"], dtype=np.float64)
    Wap = np.asarray(inputs["Wap"], dtype=np.float64)
    bap = np.asarray(inputs["bap"], dtype=np.float64)
    Wvp = np.asarray(inputs["Wvp"], dtype=np.float64)
    bvp = np.asarray(inputs["bvp"], dtype=np.float64)

    m9, s9, ctx9 = tail_partial
    m = np.concatenate([stats[..., 0].astype(np.float64), m9[None]])   # [9,B,20]
    s = np.concatenate([stats[..., 1].astype(np.float64), s9[None]])
    ctxs = np.concatenate([ctxs.astype(np.float64), ctx9[None]])       # [9,B,20,E]
    Mg = m.max(axis=0)                                   # [B,20]
    w = np.exp(m - Mg[None])
    denom = (w * s).sum(axis=0)                          # [B,20]
    ctx_full = (w[..., None] * ctxs).sum(axis=0) / denom[..., None]

    Wv_h = Wv.reshape(E, NH, DK)
    out = np.empty((B, 2, HID), np.float64)
    for ms in range(2):
        for h in range(NH):
            out[:, ms, h * DK:(h + 1) * DK] = np.einsum(
                "bd,dx->bx", ctx_full[:, ms * NH + h], Wv_h[:, h])
    out = out + bv

    mu = out.mean(-1, keepdims=True)
    var = out.var(-1, keepdims=True)
    out_ln = (out - mu) / np.sqrt(var + 1e-5) * ln_g + ln_b

    aud = out_ln[:, 0] @ Wap + bap
    vid = out_ln[:, 1] @ Wvp + bvp
    return (((aud + vid) / 2).astype(np.float32))


def run(inputs, trace=False, mode="bf16"):
    nc = _get_program()
    in_maps, tail_partial = _prepare_host(inputs)
    kw = {}
    if trace:
        _install_ntff_hook()
        import concourse.bass_utils as bu

        bu.upload_artifacts = lambda tmpdir: str(tmpdir)
        kw = dict(trace=True, trace_cores=list(range(NCORES)))
    res = run_bass_kernel_spmd(nc, in_maps, list(range(NCORES)), **kw)
    allout = np.stack([np.asarray(r["octx"], dtype=np.float64)
                       for r in res.results])                # [8,128,E+1]
    # row 32b+pair holds batch b, pair
    rows = (MPAD * np.arange(B)[:, None] + np.arange(NPAIR)[None, :])  # [B,20]
    ctxs = allout[:, rows, :E]                               # [8,B,20,E]
    stats = np.zeros((NCORES, B, NPAIR, 2), np.float64)
    stats[..., 1] = allout[:, rows, E]
    out = _finalize(inputs, ctxs, stats, tail_partial)
    return out, res


def kernel(**inputs) -> np.ndarray:
    out, _ = run(inputs, trace=False)
    return out


# revision 18
# speedup vs baseline: 1.1935x; 1.1935x over previous
"""Trainium2 Bass kernel for nn_BottlenectedAttention.

Algorithmic reduction (same as previous version): the reference consumes only
rows m=0 and m=1029 of the attention output, so per batch the whole attention
collapses to 20 logit columns ((ms, h) pairs) against an effective query
matrix  wq_eff[b] = Wq_h @ k_sel[b] / sqrt(DK)  of shape [E, 20], followed by
ctx[b, pair, :] = softmax_n(logits) @ feats[b]  and O(1)-sized host math.

This version is a ground-up rewrite of the device schedule:

* raw bass Block (no TileContext) -> no multi-round drain/EVSEM barrier tail
  (the old kernel spent ~8.7us there) and no per-instruction scheduling slack.
* Host pre-packs every tensor in its exact SBUF layout (feats^T for the logits
  matmul, natural feats for the context matmul, pos-encoding folded in) so
  each input is ONE large contiguous-per-partition DMA (2KB lines) instead of
  thousands of 512B descriptors.
* The 4 batches are packed into the 4 column-groups of the PE array
  (tile_position=(0, 32b)), so logits and context matmuls for all batches run
  concurrently in the systolic array.
* exp + per-row softmax denominators in one ScalarE activation (accum_out).
* Only 2 PE transposes (p -> p^T) + 1 DVE copy; context matmul streams the
  natural-layout feats directly.

Sharding: sequence dim, 256 rows per core x 8 cores; rows [2048, 2054) are a
host-side 9th flash shard (exactly as before).
"""
import sys

sys.path.insert(0, "/opt/trn_rl_repo")

import numpy as np

import concourse.bass as bass
import concourse.bacc as bacc
from concourse import mybir
from concourse.bass_utils import run_bass_kernel_spmd

E, HID, NH, DK, BTNK = 512, 640, 10, 64, 4
B, LA, LV = 4, 1024, 1024
L = LA + 1 + BTNK + LV + 1          # 2054
NPAIR = 2 * NH                       # 20 (ms, h) pairs per batch
MPAD = 32                            # col-group stride (batch b -> cols 32b..)
NCORES = 8
NSL = 256                            # per-core slice width
NKT = E // 128                       # 4 k-tiles over the embedding dim
NNB = NSL // 128                     # 2 n-tiles of 128
WARM_MM = 8                          # HAM warmup matmuls during input DMA
SLICES = [(c * NSL, c * NSL + NSL) for c in range(NCORES)]

F32 = mybir.dt.float32
BF16 = mybir.dt.bfloat16


def _pos_encoding(Ln, d):
    pos = np.arange(Ln, dtype=np.float32)[:, None]
    div = np.exp(np.arange(0, d, 2, dtype=np.float32) * (-np.log(10000.0) / d))
    pe = np.zeros((Ln, d), dtype=np.float32)
    pe[:, 0::2] = np.sin(pos * div).astype(np.float32)
    pe[:, 1::2] = np.cos(pos * div).astype(np.float32)
    return pe


WQ0 = 0                  # wq columns [0, 512)
ID0 = E                  # identity columns [512, 640)
FT0 = E + 128            # feats^T columns [640, 4736)
FN0 = FT0 + NKT * B * NSL   # natural feats columns [4736, 8840)
NIN = FN0 + B * NNB * E     # 8840 total input columns


def build_program():
    nc = bacc.Bacc()

    # One merged input tensor in exact SBUF layout -> two large HWDGE DMAs on
    # the sync ring (FIFO, full bandwidth, no SWDGE descriptor emission).
    in_d = nc.declare_dram_parameter("inD", [128, NIN], BF16, isOutput=False)
    out_d = nc.declare_dram_parameter("octx", [128, E + 1], BF16,
                                      isOutput=True)

    from contextlib import ExitStack
    with ExitStack() as st:
        ec = st.enter_context
        # SBUF
        inA = ec(nc.sbuf_tensor("inA", [128, NIN], BF16))
        p_s = ec(nc.sbuf_tensor("p_s", [128, NSL], BF16))
        pT_s = ec(nc.sbuf_tensor("pT_s", [128, NSL], BF16))
        s_s = ec(nc.sbuf_tensor("s_s", [128, 1], F32))
        scr_s = ec(nc.sbuf_tensor("scr_s", [128, 1], F32))
        octx_s = ec(nc.sbuf_tensor("octx_s", [128, E + 1], BF16))
        warm_s = ec(nc.sbuf_tensor("warm_s", [128, E], BF16))
        # PSUM (bank granular allocations)
        ps_w = ec(nc.psum_tensor("ps_w", [128, E], F32))
        ps_l = ec(nc.psum_tensor("ps_l", [128, E], F32))      # use [:, :NSL]
        ps_t = ec(nc.psum_tensor("ps_t", [128, 2 * NSL], BF16))  # use [:, :NSL]
        ps_c = ec(nc.psum_tensor("ps_c", [128, E], F32))
        # semaphores
        semA = ec(nc.semaphore("semA"))   # wq+ident+ftT landed
        semB = ec(nc.semaphore("semB"))   # fN landed
        osem = ec(nc.semaphore("osem"))   # output DMA landed
        vsem = ec(nc.semaphore("vsem"))   # vector milestones
        esem = ec(nc.semaphore("esem"))   # scalar: exp done
        tsem = ec(nc.semaphore("tsem"))   # PE milestones
        csem = ec(nc.semaphore("csem"))   # ctx psum->sbuf copy

        ident = inA[:, ID0:ID0 + 128]

        def wq(et, b):
            o = WQ0 + et * 128 + 32 * b
            return inA[:, o:o + MPAD]

        def ftT(et, b):
            o = FT0 + (et * B + b) * NSL
            return inA[:, o:o + NSL]

        def fN(b, nb):
            o = FN0 + (b * NNB + nb) * E
            return inA[:, o:o + E]

        with nc.Block("bk") as block:

            @block.sync
            def _(sync):
                # D1: wq + identity + feats^T; D2: natural feats.  Same HWDGE
                # ring -> FIFO, so D1 gets full bandwidth first.
                sync.dma_start(out=inA[:, :FN0],
                               in_=in_d[:, :FN0]).then_inc(semA, 16)
                sync.dma_start(out=inA[:, FN0:],
                               in_=in_d[:, FN0:]).then_inc(semB, 16)
                sync.wait_ge(csem, 1)
                sync.dma_start(out=out_d[:], in_=octx_s[:]).then_inc(osem, 16)
                # ensure the output DMA has landed before the queue retires
                sync.wait_ge(osem, 16)

            @block.gpsimd
            def _(gpsimd):
                # no gpsimd work (Pool DMA is SWDGE -> slow); empty body still
                # routes Pool to bk_end so the exit barrier completes.
                pass

            @block.tensor
            def _(tensor):
                # HAM warmup while inputs stream (~3.4us of PE activity)
                tensor.wait_ge(vsem, 1)
                for _ in range(WARM_MM):
                    tensor.matmul(ps_w[:, :E], warm_s[:, :128], warm_s[:, :E],
                                  start=True, stop=True)
                # logits: batches in col-groups, accumulate over k-tiles
                tensor.wait_ge(semA, 16)
                for et in range(NKT):
                    for b in range(B):
                        mm = tensor.matmul(
                            ps_l[32 * b:32 * b + MPAD, :NSL],
                            wq(et, b),
                            ftT(et, b),
                            start=(et == 0), stop=(et == NKT - 1),
                            tile_position=(0, 32 * b),
                            skip_group_check=True,
                        )
                mm.then_inc(tsem, 1)                      # logits done -> 1
                # p^T via PE transposes
                tensor.wait_ge(esem, 1)
                tensor.transpose(ps_t[:, 0:128], p_s[:, 0:128], ident)
                tensor.transpose(ps_t[:, 128:256], p_s[:, 128:256],
                                 ident).then_inc(tsem, 1)  # -> 2
                # ctx: batches in col-groups, accumulate over n-tiles
                tensor.wait_ge(vsem, 2)
                tensor.wait_ge(semB, 16)
                for b in range(B):
                    for nb in range(NNB):
                        mm = tensor.matmul(
                            ps_c[32 * b:32 * b + MPAD, :],
                            pT_s[:, 128 * nb + 32 * b:128 * nb + 32 * b + MPAD],
                            fN(b, nb),
                            start=(nb == 0), stop=(nb == NNB - 1),
                            tile_position=(0, 32 * b),
                            skip_group_check=True,
                        )
                mm.then_inc(tsem, 1)                      # ctx done -> 3

            @block.scalar
            def _(scalar):
                # dummy activation first so the act-table load happens at t=0
                scalar.activation(out=scr_s[:1, :], in_=scr_s[:1, :],
                                  func=mybir.ActivationFunctionType.Exp,
                                  bias=0.0, scale=0.0)
                scalar.wait_ge(tsem, 1)
                scalar.activation(out=p_s[:], in_=ps_l[:, :NSL],
                                  func=mybir.ActivationFunctionType.Exp,
                                  bias=0.0, scale=1.0,
                                  accum_out=s_s[:]).then_inc(esem, 1)
                # NOTE: no second PSUM reader here — concurrent DVE+ACT reads
                # of the same PSUM bank hang the core (Tile serializes these;
                # raw bass must simply avoid them).

            @block.vector
            def _(vector):
                vector.memset(warm_s[:], 0.0).then_inc(vsem, 1)
                vector.wait_ge(esem, 1)
                vector.tensor_copy(out=octx_s[:, E:E + 1], in_=s_s[:])
                vector.wait_ge(tsem, 2)
                vector.tensor_copy(out=pT_s[:], in_=ps_t[:, :NSL]).then_inc(vsem, 1)
                vector.wait_ge(tsem, 3)
                vector.tensor_copy(out=octx_s[:, 0:E],
                                   in_=ps_c[:, 0:E]).then_inc(csem, 1)

    nc.finalize()
    return nc


def _install_ntff_hook():
    """The agent image's antenv lacks axon_hooks; recreate it and register the
    ctypes NTFF profile hook against the injected libaxon_pjrt.so so that
    run_bass_kernel_spmd(trace=True) can capture HW exec times."""
    import contextlib
    import ctypes
    import types

    if "antenv.axon_hooks" in sys.modules:
        return
    mod = types.ModuleType("antenv.axon_hooks")
    state = {"hook": None}
    mod.set_axon_ntff_profile_hook = lambda h: state.__setitem__("hook", h)
    mod.get_axon_ntff_profile_hook = lambda: state["hook"]
    sys.modules["antenv.axon_hooks"] = mod
    try:
        import antenv

        antenv.axon_hooks = mod
    except ImportError:
        pass

    so_path = "/opt/axon/libaxon_pjrt.so"
    try:
        lib = ctypes.CDLL(so_path)
    except OSError:
        return
    if not hasattr(lib, "axon_start_nrt_profile"):
        return
    lib.axon_start_nrt_profile.argtypes = [
        ctypes.POINTER(ctypes.c_int64),
        ctypes.c_size_t,
    ]
    lib.axon_start_nrt_profile.restype = ctypes.c_int64
    lib.axon_stop_nrt_profile.argtypes = [ctypes.c_char_p]
    lib.axon_stop_nrt_profile.restype = ctypes.c_int64

    @contextlib.contextmanager
    def _hook(output_dir, device_ids):
        import jax

        jax.devices()
        if device_ids:
            ids = (ctypes.c_int64 * len(device_ids))(*device_ids)
            rc = lib.axon_start_nrt_profile(ids, len(device_ids))
        else:
            rc = lib.axon_start_nrt_profile(None, 0)
        if rc != 0:
            raise RuntimeError(f"axon_start_nrt_profile rc={rc}")
        try:
            yield
        finally:
            n = lib.axon_stop_nrt_profile(str(output_dir).encode())
            print(f"profile: {n} file(s) written to {output_dir}", file=sys.stderr)

    state["hook"] = _hook


_CACHE = {}


def _get_program():
    if "raw" not in _CACHE:
        _CACHE["raw"] = build_program()
    return _CACHE["raw"]


def _prepare_host(inputs):
    import ml_dtypes

    bf = ml_dtypes.bfloat16
    af = np.ascontiguousarray(np.asarray(inputs["audio_feat"], dtype=np.float32))
    vf = np.ascontiguousarray(np.asarray(inputs["video_feat"], dtype=np.float32))
    at = np.asarray(inputs["audio_tok"], dtype=np.float32)
    vt = np.asarray(inputs["video_tok"], dtype=np.float32)
    bt = np.asarray(inputs["btnk_toks"], dtype=np.float32)
    Wk = np.asarray(inputs["Wk"], dtype=np.float32)
    bk = np.asarray(inputs["bk"], dtype=np.float32)
    Wq = np.asarray(inputs["Wq"], dtype=np.float32)

    pe = _pos_encoding(L, E)

    raw = np.empty((B, L, E), np.float32)
    raw[:, :LA] = af
    raw[:, LA] = at[0, 0]
    raw[:, LA + 1:LA + 1 + BTNK] = bt[0]
    raw[:, LA + 1 + BTNK:LA + 1 + BTNK + LV] = vf
    raw[:, L - 1] = vt[0, 0]

    featsb = (raw + pe[None]).astype(bf)                     # [B, L, E]

    # effective query vectors (f64 host math, exactly as before)
    f_rows = np.stack([raw[:, 0] + pe[0], raw[:, LA + 1 + BTNK] + pe[LA + 1 + BTNK]],
                      axis=1).astype(np.float64)             # [B,2,E]
    k_sel = (f_rows @ Wk.astype(np.float64) + bk).reshape(B, 2, NH, DK)
    Wq_h = Wq.astype(np.float64).reshape(E, NH, DK)
    wq_eff = np.einsum("dhx,bmhx->bdmh", Wq_h, k_sel).reshape(B, E, NPAIR)
    wq_eff = wq_eff / np.sqrt(DK)                            # [B,E,20] f64

    # const region: wq columns [0,512) as [p, et*128 + 32b+j], then identity
    wq_pad = np.zeros((B, E, MPAD), np.float32)
    wq_pad[:, :, :NPAIR] = wq_eff.astype(np.float32)
    const_np = np.empty((128, E + 128), np.float32)
    # [b, et*128+p, j] -> [p, et*128 + 32b+j]
    const_np[:, :E] = (
        wq_pad.reshape(B, NKT, 128, MPAD)
        .transpose(2, 1, 0, 3)
        .reshape(128, E)
    )
    const_np[:, E:] = np.eye(128, dtype=np.float32)
    const_np = const_np.astype(bf)

    in_maps = []
    for c, (n0, n1) in enumerate(SLICES):
        block = featsb[:, n0:n1, :]                          # [B,NSL,E] bf16
        ftT = block.transpose(2, 0, 1).reshape(NKT, 128, B * NSL)
        # [et, p, b*NSL+j] -> [p, et*(B*NSL) + b*NSL + j]
        ftT = ftT.transpose(1, 0, 2).reshape(128, NKT * B * NSL)
        fN = block.reshape(B, NNB, 128, E).transpose(2, 0, 1, 3)
        fN = fN.reshape(128, B * NNB * E)   # [i, (b*NNB+nb)*E + e]
        inD = np.ascontiguousarray(
            np.concatenate([const_np, ftT, fN], axis=1))     # [128, NIN]
        in_maps.append({"inD": inD})

    # host 9th flash shard for rows [2048, L)
    n0 = NCORES * NSL
    tail = featsb[:, n0:L].astype(np.float64)                # [B,6,E]
    tail_logits = np.einsum("bnd,bdp->bnp", tail, wq_eff)
    m9 = tail_logits.max(axis=1)                             # [B,20]
    p9 = np.exp(tail_logits - m9[:, None, :])
    s9 = p9.sum(axis=1)                                      # [B,20]
    p9 = p9.astype(bf).astype(np.float64)
    ctx9 = np.einsum("bnp,bnd->bpd", p9, tail)               # [B,20,E]
    return in_maps, (m9, s9, ctx9)


def _finalize(inputs, ctxs, stats, tail_partial):
    """ctxs: [8,B,20,E] unnormalized local contexts; stats: [8,B,20,2] (m, s);
    tail_partial: host-computed 9th shard for rows [2048, 2054)."""
    Wv = np.asarray(inputs["Wv"], dtype=np.float64)
    bv = np.asarray(inputs["bv"], dtype=np.float64)
    ln_g = np.asarray(inputs["ln_g"], dtype=np.float64)
    ln_b = np.asarray(inputs["ln_b"], dtype=np.float64)
    Wap = np.asarray(inputs["Wap"], dtype=np.float64)
    bap = np.asarray(inputs["bap"], dtype=np.float64)
    Wvp = np.asarray(inputs["Wvp"], dtype=np.float64)
    bvp = np.asarray(inputs["bvp"], dtype=np.float64)

    m9, s9, ctx9 = tail_partial
    m = np.concatenate([stats[..., 0].astype(np.float64), m9[None]])   # [9,B,20]
    s = np.concatenate([stats[..., 1].astype(np.float64), s9[None]])
    ctxs = np.concatenate([ctxs.astype(np.float64), ctx9[None]])       # [9,B,20,E]
    Mg = m.max(axis=0)                                   # [B,20]
    w = np.exp(m - Mg[None])
    denom = (w * s).sum(axis=0)                          # [B,20]
    ctx_full = (w[..., None] * ctxs).sum(axis=0) / denom[..., None]

    Wv_h = Wv.reshape(E, NH, DK)
    out = np.empty((B, 2, HID), np.float64)
    for ms in range(2):
        for h in range(NH):
            out[:, ms, h * DK:(h + 1) * DK] = np.einsum(
                "bd,dx->bx", ctx_full[:, ms * NH + h], Wv_h[:, h])
    out = out + bv

    mu = out.mean(-1, keepdims=True)
    var = out.var(-1, keepdims=True)
    out_ln = (out - mu) / np.sqrt(var + 1e-5) * ln_g + ln_b

    aud = out_ln[:, 0] @ Wap + bap
    vid = out_ln[:, 1] @ Wvp + bvp
    return (((aud + vid) / 2).astype(np.float32))


def run(inputs, trace=False, mode="bf16"):
    nc = _get_program()
    in_maps, tail_partial = _prepare_host(inputs)
    kw = {}
    if trace:
        _install_ntff_hook()
        import concourse.bass_utils as bu

        bu.upload_artifacts = lambda tmpdir: str(tmpdir)
        kw = dict(trace=True, trace_cores=list(range(NCORES)))
    res = run_bass_kernel_spmd(nc, in_maps, list(range(NCORES)), **kw)
    allout = np.stack([np.asarray(r["octx"], dtype=np.float64)
                       for r in res.results])                # [8,128,E+1]
    # row 32b+pair holds batch b, pair
    rows = (MPAD * np.arange(B)[:, None] + np.arange(NPAIR)[None, :])  # [B,20]
    ctxs = allout[:, rows, :E]                               # [8,B,20,E]
    stats = np.zeros((NCORES, B, NPAIR, 2), np.float64)
    stats[..., 1] = allout[:, rows, E]
    out = _finalize(inputs, ctxs, stats, tail_partial)
    return out, res


def kernel(**inputs) -> np.ndarray:
    out, _ = run(inputs, trace=False)
    return out
